# revision 18
# baseline (speedup 1.0000x reference)
"""GAT message-passing kernel for 8 Trainium2 NeuronCores (Bass/Tile), v2.

Math (identical to the reference GAT, exploiting softmax invariances):
    att_e  = LeakyReLU_{0.2}( a[src_e] + b[dst_e] )   (+ const that cancels)
    s_e    = exp(att_e - 1)
    agg[n] = (sum_{e in seg n} s_e * emb[dst_e]) / (sum_{e in seg n} s_e)
    out[n] = sigmoid( agg[n] @ W_scale + b_scale )
with a = emb @ (W_scale @ W_att[:d]), b = emb @ (W_scale @ W_att[d:]).

v2 design (vs v1): fp8 aug-table rows of 256B holding [emb*32 fp8 | 1.0 fp8 |
b bf16]; one matmul per tile in "orientation B" (lhsT = one-hot*scores, rhs =
gathered rows + ones column -> psum [nodes, d+1] where col d accumulates the
segment score sums); LeakyReLU via ScalarE alpha; per-run batched vector ops;
a-broadcast built once via PE instead of per-tile DMA broadcasts.

Sharding: core c owns nodes [c*nslice, (c+1)*nslice); src-sorted edges are
contiguous per core. One SPMD program; per-core variation via input arrays.
"""

import os
import sys
import numpy as np

sys.path.insert(0, "/opt/trn_rl_repo")

LAST_EXEC_NS = None

_P = 128
_WIN = 32
_NCORES = 8
_HALF = 32768
_RN = 4            # tiles per dma_gather call


def _ceil_to(x, m):
    return -(-x // m) * m


def _host_prep(edge, n_nodes):
    """Index-only preprocessing: per-core padded tile streams + schedule."""
    E = edge.shape[0]
    src = np.asarray(edge[:, 0], dtype=np.int64)
    dst = np.asarray(edge[:, 1], dtype=np.int64)

    nslice = _ceil_to(-(-n_nodes // _NCORES), _P)    # nodes per core
    npad = max(nslice * _NCORES, _HALF + _P)         # aug table rows
    wpc = nslice // _WIN                             # windows per core
    ngrp = wpc // 4                                  # psum groups per core

    c_of = src // nslice
    sl = src - c_of * nslice
    w_of_e = sl // _WIN
    srel = sl % _WIN
    hi = (dst >= _HALF).astype(np.int64)

    cnt = np.zeros((_NCORES, wpc, 2), np.int64)
    np.add.at(cnt, (c_of, w_of_e, hi), 1)
    t_wk = -(-cnt.max(axis=0) // _P)                 # [wpc, 2]
    t_wk[:, 0] = np.maximum(t_wk[:, 0], 1)

    base = np.zeros((wpc, 2), np.int64)
    runs = []                    # (t0, ntiles, kind, [(w, off, rt_w)...])
    win_of = []
    ti = 0
    first_t = np.zeros(wpc, np.int64)
    last_t = np.zeros(wpc, np.int64)
    grp_start = []
    grp_end = []
    for g in range(ngrp):
        ws = list(range(4 * g, 4 * g + 4))
        grp_start.append(ti)
        for k in (0, 1):
            r0 = ti
            subs = []
            for w in ws:
                base[w, k] = ti
                if t_wk[w, k] > 0:
                    subs.append((w, ti - r0, int(t_wk[w, k])))
                win_of.extend([w] * int(t_wk[w, k]))
                ti += int(t_wk[w, k])
            if ti > r0:
                runs.append((r0, ti - r0, k, subs))
        for w in ws:
            first_t[w] = base[w, 0]
            last_t[w] = (base[w, 1] + t_wk[w, 1] - 1) if t_wk[w, 1] > 0 \
                else (base[w, 0] + t_wk[w, 0] - 1)
        grp_end.append(ti - 1)
    T = ti

    first_of = np.zeros(T, bool)
    last_of = np.zeros(T, bool)
    first_of[first_t] = True
    last_of[last_t] = True
    epi_of = np.full(T, -1, np.int64)
    gstart_of = np.full(T, -1, np.int64)
    for g in range(ngrp):
        epi_of[grp_end[g]] = g
        gstart_of[grp_start[g]] = g

    # per-edge placement: rank within (core, window, kind)
    key = (c_of * wpc + w_of_e) * 2 + hi
    order = np.lexsort((np.arange(E), key))
    ks = key[order]
    runstart = np.r_[0, np.flatnonzero(np.diff(ks)) + 1]
    runlen = np.diff(np.r_[runstart, E])
    rank = np.empty(E, np.int64)
    rank[order] = np.arange(E) - np.repeat(runstart, runlen)
    pos = base[w_of_e, hi] * _P + rank

    per_core = []
    for c in range(_NCORES):
        m = c_of == c
        p = pos[m]
        sr = np.full(T * _P, 33, np.int32)
        sr[p] = srel[m].astype(np.int32)
        gi = np.zeros(T * _P, np.int64)
        gi[p] = np.where(hi[m] == 1, dst[m] - _HALF, dst[m])
        gidx = gi.astype(np.int16)
        dstg = np.tile(gidx.reshape(T * 8, 16).T, (8, 1))
        per_core.append(dict(
            srcrel=np.ascontiguousarray(sr.reshape(T, _P).T),
            dstg=np.ascontiguousarray(dstg),
        ))

    maxrt = max(rn for (_, rn, _, _) in runs)
    sched = dict(T=T, nslice=nslice, npad=npad, wpc=wpc, ngrp=ngrp,
                 maxrt=maxrt, runs=runs,
                 win_of=win_of, first_of=first_of.tolist(),
                 last_of=last_of.tolist(), epi_of=epi_of.tolist(),
                 gstart_of=gstart_of.tolist())
    return per_core, sched


def _build_program(sched, nqueues=1, sim=False):
    import concourse.bass as bass
    import concourse.bacc as bacc
    import concourse.mybir as mybir
    import concourse.tile as tile
    from concourse.masks import make_identity
    from contextlib import ExitStack

    f32 = mybir.dt.float32
    bf16 = mybir.dt.bfloat16
    f8 = mybir.dt.float8e4
    i32 = mybir.dt.int32
    i16 = mybir.dt.int16
    Alu = mybir.AluOpType
    Act = mybir.ActivationFunctionType

    T = sched["T"]
    nslice = sched["nslice"]
    npad = sched["npad"]
    ngrp = sched["ngrp"]
    MAXRT = sched["maxrt"]
    runs = sched["runs"]
    win_of = sched["win_of"]
    first_of = sched["first_of"]
    last_of = sched["last_of"]
    epi_of = sched["epi_of"]
    gstart_of = sched["gstart_of"]
    D = 128
    NTILE = nslice // _P
    nc = bacc.Bacc("TRN2", target_bir_lowering=False, debug=False,
                   num_devices=_NCORES, dynamic_dma_scratch_size=32768,
                   num_swdge_queues=nqueues)

    # aug row (as bf16 cols): [0:64) = 128 fp8 bytes of emb*32, col 64 =
    # (fp8 1.0, 0x00), col 65 = b bf16, rest zero.
    aug = nc.declare_dram_parameter("aug", [npad, D], bf16, isOutput=False)
    embsl = nc.declare_dram_parameter("embsl", [_P, NTILE * D], f32,
                                     isOutput=False)
    wsc_d = nc.declare_dram_parameter("wsc", [D, D], f32, isOutput=False)
    watt_d = nc.declare_dram_parameter("watt", [2 * D, 1], f32, isOutput=False)
    bsc_d = nc.declare_dram_parameter("bsc", [D], f32, isOutput=False)
    srcrel_d = nc.declare_dram_parameter("srcrel", [_P, T], i32, isOutput=False)
    dstg_d = nc.declare_dram_parameter("dstg", [_P, 8 * T], i16, isOutput=False)
    out_d = nc.declare_dram_parameter("out", [_P, NTILE * D], f32,
                                     isOutput=True)

    ab_slice = nc.dram_tensor("ab_slice", [_P, 2 * NTILE], bf16)
    if sim:
        abG = nc.declare_dram_parameter("abG", [_NCORES * _P, 2 * NTILE],
                                        bf16, isOutput=False)
    else:
        abG = nc.dram_tensor("abG", [_NCORES * _P, 2 * NTILE], bf16,
                             addr_space="Shared")
    u_dram = nc.dram_tensor("u_scr", [2 * D], f32)

    with tile.TileContext(nc) as tc, ExitStack() as ctx:
        const = ctx.enter_context(tc.tile_pool(name="const", bufs=1))
        sb = ctx.enter_context(tc.tile_pool(name="sb", bufs=3))
        gpool = ctx.enter_context(tc.tile_pool(name="gp", bufs=16))
        spool = ctx.enter_context(tc.tile_pool(name="sp", bufs=4))
        epool = ctx.enter_context(tc.tile_pool(name="ep", bufs=3))
        ps_pro = ctx.enter_context(tc.tile_pool(name="pspro", bufs=1,
                                                space="PSUM"))
        ps_P = ctx.enter_context(tc.tile_pool(name="psP", bufs=2, space="PSUM"))
        ps_tr = ctx.enter_context(tc.tile_pool(name="pstr", bufs=1,
                                               space="PSUM"))
        ps_o = ctx.enter_context(tc.tile_pool(name="pso", bufs=1, space="PSUM"))

        # ---------------- constants ----------------
        ident = const.tile([_P, _P], f32)
        make_identity(nc, ident[:])
        iota = const.tile([_P, MAXRT * _WIN], i32)
        nc.gpsimd.iota(iota[:], pattern=[[0, MAXRT], [1, _WIN]], base=0,
                       channel_multiplier=0)
        iotab = const.tile([_P, MAXRT * _WIN], bf16)
        nc.vector.tensor_copy(iotab[:], iota[:])
        ones1 = const.tile([1, _P], bf16)
        nc.vector.memset(ones1[:], 1.0)
        negone = const.tile([_P, 1], f32)
        nc.vector.memset(negone[:], -1.0)
        wsb32 = const.tile([_P, D], f32)
        nc.sync.dma_start(out=wsb32[:], in_=wsc_d[:, :])
        wsb = const.tile([_P, D], bf16)
        nc.vector.tensor_copy(wsb[:], wsb32[:])
        brep = const.tile([_P, D], f32)
        nc.sync.dma_start(out=brep[:], in_=bsc_d[None, :].to_broadcast([_P, D]))
        w2 = const.tile([_P, 2], f32)
        nc.sync.dma_start(out=w2[:], in_=watt_d[:, 0].rearrange(
            "(two f) -> f two", two=2))

        # u = W_scale @ [wa | wb]
        wst_ps = ps_pro.tile([_P, _P], f32, tag="wst")
        nc.tensor.transpose(out=wst_ps[:], in_=wsb32[:], identity=ident[:])
        wst = sb.tile([_P, _P], f32, tag="wst_sb")
        nc.vector.tensor_copy(wst[:], wst_ps[:])
        u_ps = ps_pro.tile([_P, 2], f32, tag="ups")
        nc.tensor.matmul(u_ps[:], lhsT=wst[:], rhs=w2[:], start=True, stop=True)
        u_sb = sb.tile([_P, 2], f32, tag="u_sb")
        nc.vector.tensor_copy(u_sb[:], u_ps[:])
        nc.sync.dma_start(
            out=u_dram[:].rearrange("(j dd) -> dd j", j=2), in_=u_sb[:])
        urep = const.tile([_P, 2 * D], f32)
        nc.sync.dma_start(out=urep[:], in_=u_dram[None, :].to_broadcast(
            [_P, 2 * D]))

        # ---------------- a/b for own nodes ----------------
        esl = const.tile([_P, NTILE * D], f32)
        nc.sync.dma_start(out=esl[:], in_=embsl[:, :])
        absl = const.tile([_P, 2 * NTILE], f32)
        for t in range(NTILE):
            prod = sb.tile([_P, 2 * D], f32, tag="prod")
            nc.vector.tensor_tensor(
                out=prod[:],
                in0=esl[:, t * D:(t + 1) * D]
                    .rearrange("p (one d) -> p one d", one=1)
                    .to_broadcast([_P, 2, D]),
                in1=urep[:, :].rearrange("p (j d) -> p j d", j=2),
                op=Alu.mult)
            nc.vector.tensor_reduce(
                out=absl[:, 2 * t:2 * t + 2],
                in_=prod[:, :].rearrange("p (j d) -> p j d", j=2),
                axis=mybir.AxisListType.X, op=Alu.add)
        ab16 = const.tile([_P, 2 * NTILE], bf16)
        nc.vector.tensor_copy(ab16[:], absl[:])
        nc.sync.dma_start(out=ab_slice[:, 0:NTILE], in_=ab16[:, 0:2 * NTILE:2])
        nc.scalar.dma_start(out=ab_slice[:, NTILE:2 * NTILE],
                            in_=ab16[:, 1:2 * NTILE:2])
        if not sim:
            nc.gpsimd.collective_compute(
                "AllGather", Alu.bypass,
                replica_groups=[list(range(_NCORES))],
                ins=[ab_slice[:, :]], outs=[abG[:, :]])

        # b column (bf16 col 65) of the aug table from abG's b blocks
        dengs = [nc.sync, nc.scalar]
        with nc.allow_non_contiguous_dma(reason="2B b-column scatter"):
            for c in range(_NCORES):
                dengs[c % 2].dma_start(
                    out=aug[c * nslice:(c + 1) * nslice, 65]
                        .rearrange("(t p) -> p t", p=_P),
                    in_=abG[c * _P:(c + 1) * _P, NTILE:2 * NTILE])

        # a replicated across partitions: a_rep[p, t*128+q] = a[t*128+q],
        # built tile-by-tile on PE: row_t = absl_a[:, t].T (via identity),
        # then ones1 (x) row_t broadcast.
        a_rep = const.tile([_P, nslice], bf16)
        for t in range(NTILE):
            rp1 = ps_pro.tile([1, _P], f32, tag="arow_ps")
            nc.tensor.matmul(rp1[:], lhsT=absl[:, 2 * t:2 * t + 1],
                             rhs=ident[:], start=True, stop=True)
            row = sb.tile([1, _P], bf16, tag="arow_sb")
            nc.scalar.activation(row[:], rp1[:], Act.Copy)
            rp2 = ps_pro.tile([_P, _P], f32, tag="arep_ps")
            nc.tensor.matmul(rp2[:], lhsT=ones1[:, :], rhs=row[:],
                             start=True, stop=True)
            nc.scalar.activation(a_rep[:, t * _P:(t + 1) * _P], rp2[:],
                                 Act.Copy)

        # ---------------- index arrays ----------------
        srci = sb.tile([_P, T], i32, tag="srci")
        nc.sync.dma_start(out=srci[:], in_=srcrel_d[:, :])
        srb = const.tile([_P, T], bf16)
        nc.vector.tensor_copy(srb[:], srci[:])
        dstg = const.tile([_P, 8 * T], i16)
        nc.scalar.dma_start(out=dstg[:], in_=dstg_d[:, :])

        # ---------------- main loop ----------------
        o_stage = const.tile([_P, ngrp * D], bf16)
        P_ps = None
        call_i = 0
        for (r0, rt, rkind, subs) in runs:
            g = win_of[r0] // 4
            if gstart_of[r0] == g and first_of[r0]:
                pass  # group psum allocated below at first matmul
            src_ap = aug[0:_HALF, :] if rkind == 0 else aug[_HALF:npad, :]

            # gathers for this run
            gtiles = []
            for c0 in range(0, rt, _RN):
                rn = min(_RN, rt - c0)
                G = gpool.tile([_P, _RN * D], bf16, tag="G")
                nc.gpsimd.dma_gather(
                    out_ap=G[:, :rn * D].rearrange("p (k r) -> p k r", r=D),
                    in_ap=src_ap,
                    idxs_ap=dstg[:, 8 * (r0 + c0):8 * (r0 + c0 + rn)],
                    num_idxs=rn * _P,
                    num_idxs_reg=rn * _P,
                    elem_size=D,
                    queue_num=call_i % nqueues)
                call_i += 1
                gtiles.append((c0, rn, G))

            # b per tile of the run
            b_run = spool.tile([_P, MAXRT], bf16, tag="brun")
            for (c0, rn, G) in gtiles:
                nc.vector.tensor_copy(
                    b_run[:, c0:c0 + rn], G[:, 65:rn * D:D])

            # X[e, j*32+k] = a[w_j*32 + k] + b[e, j]
            X = spool.tile([_P, MAXRT * _WIN], bf16, tag="X")
            for (w, off, rtw) in subs:
                nc.vector.tensor_tensor(
                    out=X[:, off * _WIN:(off + rtw) * _WIN],
                    in0=a_rep[:, w * _WIN:(w + 1) * _WIN]
                        .rearrange("p (o k) -> p o k", o=1)
                        .to_broadcast([_P, rtw, _WIN]),
                    in1=b_run[:, off:off + rtw]
                        .rearrange("p (j o) -> p j o", o=1)
                        .to_broadcast([_P, rtw, _WIN]),
                    op=Alu.add)
            # S = exp(LeakyReLU_0.2(X) - 1); LR on DVE so ScalarE keeps one
            # ACT table (Exp) loaded for the whole main loop.
            V = spool.tile([_P, MAXRT * _WIN], bf16, tag="V")
            if os.environ.get("GAT_LRELU_ACT"):
                nc.scalar.activation(V[:, :rt * _WIN], X[:, :rt * _WIN],
                                     Act.Lrelu, bias=0.0, scale=1.0, alpha=0.2)
            else:
                nc.vector.tensor_scalar_mul(V[:, :rt * _WIN],
                                            X[:, :rt * _WIN], 0.2)
                nc.vector.tensor_tensor(out=V[:, :rt * _WIN],
                                        in0=V[:, :rt * _WIN],
                                        in1=X[:, :rt * _WIN], op=Alu.max)
            S = spool.tile([_P, MAXRT * _WIN], bf16, tag="S")
            nc.scalar.activation(S[:, :rt * _WIN], V[:, :rt * _WIN],
                                 Act.Exp, bias=negone[:, 0:1], scale=1.0)
            # oh = onehot(srcrel), so = oh * S
            oh = spool.tile([_P, MAXRT * _WIN], bf16, tag="oh")
            nc.vector.tensor_tensor(
                out=oh[:, :rt * _WIN],
                in0=srb[:, r0:r0 + rt]
                    .rearrange("p (k o) -> p k o", o=1)
                    .to_broadcast([_P, rt, _WIN]),
                in1=iotab[:, :rt * _WIN].rearrange("p (k w) -> p k w", w=_WIN),
                op=Alu.is_equal)
            so = spool.tile([_P, MAXRT * _WIN], f8, tag="so")
            nc.vector.tensor_tensor(
                out=so[:, :rt * _WIN], in0=oh[:, :rt * _WIN],
                in1=S[:, :rt * _WIN], op=Alu.mult)

            # matmuls
            for (c0, rn, G) in gtiles:
                G8 = G[:, :].bitcast(f8)
                for j in range(rn):
                    t = r0 + c0 + j
                    w = win_of[t]
                    j4 = w % 4
                    if first_of[t] and j4 == 0 and rkind == 0:
                        P_ps = ps_P.tile([_P, D + 1], f32, tag="P")
                    nc.tensor.matmul(
                        P_ps[j4 * _WIN:(j4 + 1) * _WIN, 0:D + 1],
                        lhsT=so[:, (c0 + j) * _WIN:(c0 + j + 1) * _WIN],
                        rhs=G8[:, j * 2 * D:j * 2 * D + D + 1],
                        start=first_of[t], stop=last_of[t],
                        tile_position=(0, j4 * _WIN),
                        skip_group_check=True)

                    g_epi = epi_of[t]
                    if g_epi >= 0:
                        # epilogue for group g_epi
                        inv = epool.tile([_P, 1], f32, tag="inv")
                        sclamp = epool.tile([_P, 1], f32, tag="scl")
                        nc.vector.tensor_scalar_max(
                            sclamp[:], P_ps[:, D:D + 1], 1e-30)
                        nc.vector.reciprocal(inv[:], sclamp[:])
                        aggsc = epool.tile([_P, D], f32, tag="aggsc")
                        nc.vector.tensor_scalar(
                            out=aggsc[:], in0=P_ps[:, 0:D],
                            scalar1=inv[:, 0:1], scalar2=1.0 / 32.0,
                            op0=Alu.mult, op1=Alu.mult)
                        trp = ps_tr.tile([_P, D], f32, tag="trp")
                        nc.tensor.transpose(out=trp[:], in_=aggsc[:],
                                            identity=ident[:])
                        aggT = epool.tile([_P, D], bf16, tag="aggT")
                        nc.scalar.activation(aggT[:], trp[:], Act.Copy)
                        o_ps = ps_o.tile([_P, D], f32, tag="ops")
                        nc.tensor.matmul(o_ps[:], lhsT=aggT[:], rhs=wsb[:],
                                         start=True, stop=True)
                        nc.vector.tensor_tensor(
                            out=o_stage[:, g_epi * D:(g_epi + 1) * D],
                            in0=o_ps[:], in1=brep[:], op=Alu.add)

        # ---------------- final sigmoid + output ----------------
        for b0 in range(0, ngrp, 4):
            bn = min(4, ngrp - b0)
            of = epool.tile([_P, 4 * D], f32, tag="ofin")
            nc.scalar.activation(of[:, :bn * D],
                                 o_stage[:, b0 * D:(b0 + bn) * D], Act.Sigmoid)
            nc.sync.dma_start(out=out_d[:, b0 * D:(b0 + bn) * D],
                              in_=of[:, :bn * D])

    nc.finalize()
    return nc


def _pack_aug(emb_f32, npad, n_nodes):
    import ml_dtypes
    raw = np.zeros((npad, 256), np.uint8)
    scaled = (emb_f32 * 32.0).astype(ml_dtypes.float8_e4m3)
    raw[:n_nodes, 0:128] = scaled.view(np.uint8)
    one = np.array(1.0, ml_dtypes.float8_e4m3).reshape(1).view(np.uint8)[0]
    raw[:, 128] = one
    return np.ascontiguousarray(raw.view(ml_dtypes.bfloat16))


def kernel(edge, emb_mat, W_scale, b_scale, W_att, b_att):
    global LAST_EXEC_NS
    from concourse.bass_utils import run_bass_kernel_spmd

    n_nodes, d = emb_mat.shape
    assert d == 128
    per_core, sched = _host_prep(np.asarray(edge), n_nodes)

    nslice, npad = sched["nslice"], sched["npad"]
    emb_f32 = np.asarray(emb_mat, np.float32)
    aug = _pack_aug(emb_f32, npad, n_nodes)
    emb_pad = np.zeros((_NCORES * nslice, 128), np.float32)
    emb_pad[:n_nodes] = emb_f32
    wsc = np.ascontiguousarray(np.asarray(W_scale, np.float32))
    watt = np.ascontiguousarray(np.asarray(W_att, np.float32).reshape(256, 1))
    bsc = np.ascontiguousarray(np.asarray(b_scale, np.float32).reshape(128))

    nqueues = int(os.environ.get("GAT_QUEUES", "4"))
    nc = _build_program(sched, nqueues=nqueues)

    ntile = nslice // _P
    in_maps = []
    for c in range(_NCORES):
        esl = emb_pad[c * nslice:(c + 1) * nslice]
        esl = esl.reshape(ntile, _P, 128).transpose(1, 0, 2).reshape(
            _P, ntile * 128)
        in_maps.append({
            "aug": aug,
            "embsl": np.ascontiguousarray(esl),
            "wsc": wsc, "watt": watt, "bsc": bsc,
            "srcrel": per_core[c]["srcrel"],
            "dstg": per_core[c]["dstg"],
        })

    trace = bool(int(os.environ.get("GAT_PROFILE", "0")))
    if trace:
        _install_profile_shim()
    res = run_bass_kernel_spmd(nc, in_maps, core_ids=list(range(_NCORES)),
                               trace=trace)
    LAST_EXEC_NS = res.exec_time_ns
    parts = []
    for c in range(_NCORES):
        o = res.results[c]["out"]
        parts.append(o.reshape(_P, ntile, 128).transpose(1, 0, 2)
                     .reshape(nslice, 128))
    out = np.concatenate(parts, axis=0)
    return out[:n_nodes]


def _install_profile_shim():
    """Register the NTFF profile hook if the image didn't (test-time only)."""
    import types
    try:
        import antenv.axon_hooks  # noqa: F401
        return
    except ImportError:
        pass
    try:
        from trn_agent_boot.trn_boot import _ntff_profile_via_ctypes
        hook = _ntff_profile_via_ctypes("/opt/axon/libaxon_pjrt.so")
        mod = types.ModuleType("antenv.axon_hooks")
        mod.get_axon_ntff_profile_hook = lambda: hook
        sys.modules["antenv.axon_hooks"] = mod
    except Exception:
        pass
